# revision 5
# baseline (speedup 1.0000x reference)
"""Trainium2 Bass kernel for nn_MoELayer (top-2 MoE, E=8 experts).

Strategy (expert-parallel across 8 NeuronCores):
  - Host computes the (tiny) gate matmul + top-2 + softmax, and dispatches
    each token to its two experts' cores ("all-to-all" done host-side as the
    sharding step). One expert per core.
  - Each core runs a Bass kernel computing, for its expert e and its routed
    tokens:   out = (silu(tok @ W1[e]) @ W2[e]) * gate_weight
    with bf16 matmul inputs and fp32 PSUM accumulation. Weights stay
    resident in SBUF; only the top-2-selected tokens are computed
    (4x fewer FLOPs than the dense reference).
  - Host scatter-adds the two weighted expert outputs per token.

Layouts (chosen so no on-device transposes are needed):
  stage 1:  actT[f, c] = silu( sum_d W1[d, f] * tokT[d, c] )
            matmul(lhsT=W1[dk, fj-tile], rhs=tokT[dk, c-chunk]) -> PSUM [f, c]
  stage 2:  out[c, d] = sum_f actT[f, c] * W2[f, d]
            matmul(lhsT=actT[fk, c-tile], rhs=W2[fk, d-chunk]) -> PSUM [c, d]
"""

import math
import sys

sys.path.insert(0, "/opt/trn_rl_repo")

import ml_dtypes
import numpy as np

B, T, D, F, E = 2, 2048, 1024, 4096, 8
N = B * T
P = 128
KD = D // P  # 8
KF = F // P  # 32

bf16 = ml_dtypes.bfloat16

_nc_cache: dict[int, object] = {}
LAST_RESULTS = None  # BassKernelResults from the most recent run (for test.py)
TRACE = False


def _chunk_sizes(C: int) -> list[int]:
    """Split C (multiple of 128) into near-equal chunks of <=512, each a
    multiple of 128, so every stage-1 matmul has a large free dim."""
    n = math.ceil(C / 512)
    base = math.ceil(C / (n * P)) * P
    sizes = []
    rem = C
    while rem > 0:
        s = min(base, rem)
        sizes.append(s)
        rem -= s
    return sizes


def _build(C: int):
    import concourse.mybir as mybir
    import concourse.tile as tile
    from concourse import bacc
    from concourse.tile_rust import add_dep_helper

    dt = mybir.dt
    CM = C // P

    nc = bacc.Bacc(None, target_bir_lowering=False)

    tokt = nc.dram_tensor("tokt", [P, KD, C], dt.bfloat16, kind="ExternalInput")
    w1 = nc.dram_tensor("w1", [P, KD, F], dt.bfloat16, kind="ExternalInput")
    w2 = nc.dram_tensor("w2", [P, KF, D], dt.bfloat16, kind="ExternalInput")
    wt = nc.dram_tensor("wt", [P, CM], dt.float32, kind="ExternalInput")
    out = nc.dram_tensor("out", [C, D], dt.float32, kind="ExternalOutput")

    chunks = _chunk_sizes(C)

    with tile.TileContext(nc) as tc:
        with (
            tc.tile_pool(name="const", bufs=1) as cpool,
            tc.tile_pool(name="act", bufs=1) as apool,
            tc.tile_pool(name="ps1", bufs=2, space="PSUM") as ps1pool,
            tc.tile_pool(name="ps2", bufs=2, space="PSUM") as ps2pool,
            tc.tile_pool(name="warm", bufs=1, space="PSUM") as wpool,
            tc.tile_pool(name="ob", bufs=4) as opool,
        ):
            w1_sb = cpool.tile([P, KD, F], dt.bfloat16, tag="w1")
            w2_sb = cpool.tile([P, KF, D], dt.bfloat16, tag="w2")
            tok_sb = cpool.tile([P, KD, C], dt.bfloat16, tag="tok")
            wt_sb = cpool.tile([P, CM], dt.float32, tag="wt")

            # ---- PE warm-up: dummy matmuls with no DMA dependency so the
            # HAM clock-gate reaches K=8/8 before the real stream starts,
            # covering the input-DMA window with (discarded) PE work.
            wa = cpool.tile([P, P], dt.bfloat16, tag="wa")
            wb = cpool.tile([P, 192], dt.bfloat16, tag="wb")
            nc.vector.memset(wa[:], 0.0)
            nc.vector.memset(wb[:], 0.0)
            warm_ps = wpool.tile([P, 192], dt.float32, tag="warm")
            for _ in range(40):
                nc.tensor.matmul(warm_ps[:], wa[:], wb[:], start=True, stop=True)

            # ---- input DMAs: few big transfers (each fans out across HW
            # queues at full bandwidth), dependency-chained in priority
            # order so the first-needed data never waits behind later data.
            c0s = [sum(chunks[:i]) for i in range(len(chunks))]
            chain = []
            chain.append(
                nc.sync.dma_start(
                    tok_sb[:, :, c0s[0] : c0s[0] + chunks[0]],
                    tokt[:, :, c0s[0] : c0s[0] + chunks[0]],
                )
            )
            FQ = F // 4
            for j in range(4):
                chain.append(
                    nc.sync.dma_start(
                        w1_sb[:, :, j * FQ : (j + 1) * FQ],
                        w1[:, :, j * FQ : (j + 1) * FQ],
                    )
                )
            for i in range(1, len(chunks)):
                chain.append(
                    nc.sync.dma_start(
                        tok_sb[:, :, c0s[i] : c0s[i] + chunks[i]],
                        tokt[:, :, c0s[i] : c0s[i] + chunks[i]],
                    )
                )
            chain.append(nc.sync.dma_start(wt_sb[:], wt[:]))
            for a, b in zip(chain, chain[1:]):
                add_dep_helper(a.ins, b.ins, sync=True, reason="dma priority chain")

            first_chunk = True
            c0 = 0
            for cn in chunks:
                act_sb = apool.tile([P, KF, cn], dt.bfloat16, tag="act")
                # ---- stage 1: actT = silu(W1^T @ tokT) ----
                for fj in range(KF):
                    ps1 = ps1pool.tile([P, cn], dt.float32, tag="ps1")
                    first_mm = None
                    for dk in range(KD):
                        mm = nc.tensor.matmul(
                            ps1[:],
                            w1_sb[:, dk, fj * P : (fj + 1) * P],
                            tok_sb[:, dk, c0 : c0 + cn],
                            start=(dk == 0),
                            stop=(dk == KD - 1),
                        )
                        if first_mm is None:
                            first_mm = mm
                    nc.scalar.activation(
                        act_sb[:, fj, :],
                        ps1[:],
                        mybir.ActivationFunctionType.Silu,
                    )
                    # defer the W2 halves until stage 1 is well underway so
                    # the early HBM bandwidth all goes to W1/tokens
                    if first_chunk and fj in (8, 16):
                        q = 0 if fj == 8 else 1
                        dma = nc.sync.dma_start(
                            w2_sb[:, q * (KF // 2) : (q + 1) * (KF // 2), :],
                            w2[:, q * (KF // 2) : (q + 1) * (KF // 2), :],
                        )
                        add_dep_helper(
                            first_mm.ins,
                            dma.ins,
                            sync=True,
                            reason="defer w2 load behind stage-1 progress",
                        )
                # ---- stage 2: out = (actT^T @ W2) * wt ----
                for cm in range(cn // P):
                    col = c0 // P + cm
                    for dn in range(D // 512):
                        ps2 = ps2pool.tile([P, 512], dt.float32, tag="ps2")
                        for fk in range(KF):
                            nc.tensor.matmul(
                                ps2[:],
                                act_sb[:, fk, cm * P : (cm + 1) * P],
                                w2_sb[:, fk, dn * 512 : (dn + 1) * 512],
                                start=(fk == 0),
                                stop=(fk == KF - 1),
                            )
                        ob = opool.tile([P, 512], dt.float32, tag="ob")
                        nc.vector.tensor_scalar_mul(
                            ob[:], ps2[:], wt_sb[:, col : col + 1]
                        )
                        nc.sync.dma_start(
                            out[
                                c0 + cm * P : c0 + (cm + 1) * P,
                                dn * 512 : (dn + 1) * 512,
                            ],
                            ob[:],
                        )
                c0 += cn
                first_chunk = False

    nc.compile()
    return nc


def _get_nc(C: int):
    if C not in _nc_cache:
        _nc_cache[C] = _build(C)
    return _nc_cache[C]


def kernel(**inputs) -> np.ndarray:
    global LAST_RESULTS
    x = np.asarray(inputs["x"], dtype=np.float32)
    Wg = np.asarray(inputs["Wg"], dtype=np.float32)
    W1 = np.asarray(inputs["W1"], dtype=np.float32)
    W2 = np.asarray(inputs["W2"], dtype=np.float32)

    h = np.ascontiguousarray(x.reshape(N, D))

    # ---- host gate: top-2 + softmax (0.05% of total FLOPs) ----
    logits = h @ Wg.T  # [N, E] f32
    idx2 = np.argpartition(-logits, 1, axis=1)[:, :2]
    lsel = np.take_along_axis(logits, idx2, axis=1)
    first = lsel[:, 0] >= lsel[:, 1]
    i0 = np.where(first, idx2[:, 0], idx2[:, 1])
    i1 = np.where(first, idx2[:, 1], idx2[:, 0])
    l0 = np.where(first, lsel[:, 0], lsel[:, 1])
    l1 = np.where(first, lsel[:, 1], lsel[:, 0])
    e1 = np.exp((l1 - l0).astype(np.float32))
    w0 = (1.0 / (1.0 + e1)).astype(np.float32)
    w1g = (e1 / (1.0 + e1)).astype(np.float32)

    token_ids = np.concatenate([np.arange(N), np.arange(N)])
    expert_ids = np.concatenate([i0, i1])
    gate_w = np.concatenate([w0, w1g])

    counts = np.bincount(expert_ids, minlength=E)
    C = max(P, int(math.ceil(counts.max() / P)) * P)
    CM = C // P

    hb = h.astype(bf16)
    W1b = W1.astype(bf16)
    W2b = W2.astype(bf16)

    in_maps = []
    ids_per_expert = []
    for e in range(E):
        sel = np.flatnonzero(expert_ids == e)
        ids_e = token_ids[sel]
        n_e = len(ids_e)
        ids_per_expert.append(ids_e)

        tokT = np.zeros((P, KD, C), dtype=bf16)
        # tokens [n,D] -> [D,n] -> [KD,P,n] -> [P,KD,n]
        tokT[:, :, :n_e] = (
            hb[ids_e].T.reshape(KD, P, n_e).transpose(1, 0, 2)
        )
        wt_e = np.zeros((C,), dtype=np.float32)
        wt_e[:n_e] = gate_w[sel]
        in_maps.append(
            {
                "tokt": tokT,
                "w1": np.ascontiguousarray(
                    W1b[e].reshape(KD, P, F).transpose(1, 0, 2)
                ),
                "w2": np.ascontiguousarray(
                    W2b[e].reshape(KF, P, D).transpose(1, 0, 2)
                ),
                "wt": np.ascontiguousarray(wt_e.reshape(CM, P).T),
            }
        )

    nc = _get_nc(C)
    from concourse.bass_utils import run_bass_kernel_spmd

    LAST_RESULTS = run_bass_kernel_spmd(
        nc, in_maps, core_ids=list(range(E)), trace=TRACE
    )

    y = np.zeros((N, D), dtype=np.float32)
    for e in range(E):
        o = np.asarray(LAST_RESULTS.results[e]["out"], dtype=np.float32)
        ids_e = ids_per_expert[e]
        y[ids_e] += o[: len(ids_e)]
    return y.reshape(B, T, D)


# revision 6
# speedup vs baseline: 1.1367x; 1.1367x over previous
"""Trainium2 Bass kernel for nn_MoELayer (top-2 MoE, E=8 experts).

Strategy (expert-parallel across 8 NeuronCores):
  - Host computes the (tiny) gate matmul + top-2 + softmax, and dispatches
    each token to its two experts' cores ("all-to-all" done host-side as the
    sharding step). One expert per core.
  - Each core runs a Bass kernel computing, for its expert e and its routed
    tokens:   out = (silu(tok @ W1[e]) @ W2[e]) * gate_weight
    with bf16 matmul inputs and fp32 PSUM accumulation. Weights stay
    resident in SBUF; only the top-2-selected tokens are computed
    (4x fewer FLOPs than the dense reference).
  - Host scatter-adds the two weighted expert outputs per token.

Layouts (chosen so no on-device transposes are needed):
  stage 1:  actT[f, c] = silu( sum_d W1[d, f] * tokT[d, c] )
            matmul(lhsT=W1[dk, fj-tile], rhs=tokT[dk, c-chunk]) -> PSUM [f, c]
  stage 2:  out[c, d] = sum_f actT[f, c] * W2[f, d]
            matmul(lhsT=actT[fk, c-tile], rhs=W2[fk, d-chunk]) -> PSUM [c, d]
"""

import math
import sys

sys.path.insert(0, "/opt/trn_rl_repo")

import ml_dtypes
import numpy as np

B, T, D, F, E = 2, 2048, 1024, 4096, 8
N = B * T
P = 128
KD = D // P  # 8
KF = F // P  # 32

bf16 = ml_dtypes.bfloat16

_nc_cache: dict[int, object] = {}
LAST_RESULTS = None  # BassKernelResults from the most recent run (for test.py)
TRACE = False


def _chunk_sizes(C: int) -> list[int]:
    """Split C (multiple of 128) into near-equal chunks of <=512, each a
    multiple of 128, so every stage-1 matmul has a large free dim."""
    n = math.ceil(C / 512)
    base = math.ceil(C / (n * P)) * P
    sizes = []
    rem = C
    while rem > 0:
        s = min(base, rem)
        sizes.append(s)
        rem -= s
    return sizes


def _build(C: int):
    import concourse.mybir as mybir
    import concourse.tile as tile
    from concourse import bacc
    from concourse.tile_rust import add_dep_helper

    dt = mybir.dt
    CM = C // P

    nc = bacc.Bacc(None, target_bir_lowering=False)

    tokt = nc.dram_tensor("tokt", [P, KD, C], dt.bfloat16, kind="ExternalInput")
    w1 = nc.dram_tensor("w1", [P, KD, F], dt.bfloat16, kind="ExternalInput")
    w2 = nc.dram_tensor("w2", [P, KF, D], dt.bfloat16, kind="ExternalInput")
    wt = nc.dram_tensor("wt", [P, CM], dt.float32, kind="ExternalInput")
    out = nc.dram_tensor("out", [C, D], dt.float32, kind="ExternalOutput")

    chunks = _chunk_sizes(C)

    with tile.TileContext(nc) as tc:
        with (
            tc.tile_pool(name="const", bufs=1) as cpool,
            tc.tile_pool(name="act", bufs=1) as apool,
            tc.tile_pool(name="ps1", bufs=2, space="PSUM") as ps1pool,
            tc.tile_pool(name="ps2", bufs=2, space="PSUM") as ps2pool,
            tc.tile_pool(name="warm", bufs=1, space="PSUM") as wpool,
            tc.tile_pool(name="ob", bufs=4) as opool,
        ):
            w1_sb = cpool.tile([P, KD, F], dt.bfloat16, tag="w1")
            w2_sb = cpool.tile([P, KF, D], dt.bfloat16, tag="w2")
            tok_sb = cpool.tile([P, KD, C], dt.bfloat16, tag="tok")
            wt_sb = cpool.tile([P, CM], dt.float32, tag="wt")

            # ---- PE warm-up: dummy matmuls with no DMA dependency so the
            # HAM clock-gate reaches K=8/8 before the real stream starts,
            # covering the input-DMA window with (discarded) PE work.
            wa = cpool.tile([P, P], dt.bfloat16, tag="wa")
            wb = cpool.tile([P, 192], dt.bfloat16, tag="wb")
            nc.vector.memset(wa[:], 0.0)
            nc.vector.memset(wb[:], 0.0)
            warm_ps = wpool.tile([P, 192], dt.float32, tag="warm")
            for _ in range(40):
                nc.tensor.matmul(warm_ps[:], wa[:], wb[:], start=True, stop=True)

            # ---- input DMAs: concurrent big transfers; W2 alone is
            # deferred (below) so early HBM bandwidth goes to W1/tokens.
            c0s = [sum(chunks[:i]) for i in range(len(chunks))]
            for i in range(len(chunks)):
                nc.sync.dma_start(
                    tok_sb[:, :, c0s[i] : c0s[i] + chunks[i]],
                    tokt[:, :, c0s[i] : c0s[i] + chunks[i]],
                )
            FQ = F // 4
            for j in range(4):
                nc.sync.dma_start(
                    w1_sb[:, :, j * FQ : (j + 1) * FQ],
                    w1[:, :, j * FQ : (j + 1) * FQ],
                )
            nc.sync.dma_start(wt_sb[:], wt[:])

            first_chunk = True
            c0 = 0
            for cn in chunks:
                act_sb = apool.tile([P, KF, cn], dt.bfloat16, tag="act")
                # ---- stage 1: actT = silu(W1^T @ tokT) ----
                for fj in range(KF):
                    ps1 = ps1pool.tile([P, cn], dt.float32, tag="ps1")
                    first_mm = None
                    for dk in range(KD):
                        mm = nc.tensor.matmul(
                            ps1[:],
                            w1_sb[:, dk, fj * P : (fj + 1) * P],
                            tok_sb[:, dk, c0 : c0 + cn],
                            start=(dk == 0),
                            stop=(dk == KD - 1),
                        )
                        if first_mm is None:
                            first_mm = mm
                    nc.scalar.activation(
                        act_sb[:, fj, :],
                        ps1[:],
                        mybir.ActivationFunctionType.Silu,
                    )
                    # defer the W2 halves until stage 1 is well underway so
                    # the early HBM bandwidth all goes to W1/tokens
                    if first_chunk and fj in (8, 16):
                        q = 0 if fj == 8 else 1
                        dma = nc.sync.dma_start(
                            w2_sb[:, q * (KF // 2) : (q + 1) * (KF // 2), :],
                            w2[:, q * (KF // 2) : (q + 1) * (KF // 2), :],
                        )
                        add_dep_helper(
                            first_mm.ins,
                            dma.ins,
                            sync=True,
                            reason="defer w2 load behind stage-1 progress",
                        )
                # ---- stage 2: out = (actT^T @ W2) * wt ----
                for cm in range(cn // P):
                    col = c0 // P + cm
                    for dn in range(D // 512):
                        ps2 = ps2pool.tile([P, 512], dt.float32, tag="ps2")
                        for fk in range(KF):
                            nc.tensor.matmul(
                                ps2[:],
                                act_sb[:, fk, cm * P : (cm + 1) * P],
                                w2_sb[:, fk, dn * 512 : (dn + 1) * 512],
                                start=(fk == 0),
                                stop=(fk == KF - 1),
                            )
                        ob = opool.tile([P, 512], dt.float32, tag="ob")
                        nc.vector.tensor_scalar_mul(
                            ob[:], ps2[:], wt_sb[:, col : col + 1]
                        )
                        nc.sync.dma_start(
                            out[
                                c0 + cm * P : c0 + (cm + 1) * P,
                                dn * 512 : (dn + 1) * 512,
                            ],
                            ob[:],
                        )
                c0 += cn
                first_chunk = False

    nc.compile()
    return nc


def _get_nc(C: int):
    if C not in _nc_cache:
        _nc_cache[C] = _build(C)
    return _nc_cache[C]


def kernel(**inputs) -> np.ndarray:
    global LAST_RESULTS
    x = np.asarray(inputs["x"], dtype=np.float32)
    Wg = np.asarray(inputs["Wg"], dtype=np.float32)
    W1 = np.asarray(inputs["W1"], dtype=np.float32)
    W2 = np.asarray(inputs["W2"], dtype=np.float32)

    h = np.ascontiguousarray(x.reshape(N, D))

    # ---- host gate: top-2 + softmax (0.05% of total FLOPs) ----
    logits = h @ Wg.T  # [N, E] f32
    idx2 = np.argpartition(-logits, 1, axis=1)[:, :2]
    lsel = np.take_along_axis(logits, idx2, axis=1)
    first = lsel[:, 0] >= lsel[:, 1]
    i0 = np.where(first, idx2[:, 0], idx2[:, 1])
    i1 = np.where(first, idx2[:, 1], idx2[:, 0])
    l0 = np.where(first, lsel[:, 0], lsel[:, 1])
    l1 = np.where(first, lsel[:, 1], lsel[:, 0])
    e1 = np.exp((l1 - l0).astype(np.float32))
    w0 = (1.0 / (1.0 + e1)).astype(np.float32)
    w1g = (e1 / (1.0 + e1)).astype(np.float32)

    token_ids = np.concatenate([np.arange(N), np.arange(N)])
    expert_ids = np.concatenate([i0, i1])
    gate_w = np.concatenate([w0, w1g])

    counts = np.bincount(expert_ids, minlength=E)
    C = max(P, int(math.ceil(counts.max() / P)) * P)
    CM = C // P

    hb = h.astype(bf16)
    W1b = W1.astype(bf16)
    W2b = W2.astype(bf16)

    in_maps = []
    ids_per_expert = []
    for e in range(E):
        sel = np.flatnonzero(expert_ids == e)
        ids_e = token_ids[sel]
        n_e = len(ids_e)
        ids_per_expert.append(ids_e)

        tokT = np.zeros((P, KD, C), dtype=bf16)
        # tokens [n,D] -> [D,n] -> [KD,P,n] -> [P,KD,n]
        tokT[:, :, :n_e] = (
            hb[ids_e].T.reshape(KD, P, n_e).transpose(1, 0, 2)
        )
        wt_e = np.zeros((C,), dtype=np.float32)
        wt_e[:n_e] = gate_w[sel]
        in_maps.append(
            {
                "tokt": tokT,
                "w1": np.ascontiguousarray(
                    W1b[e].reshape(KD, P, F).transpose(1, 0, 2)
                ),
                "w2": np.ascontiguousarray(
                    W2b[e].reshape(KF, P, D).transpose(1, 0, 2)
                ),
                "wt": np.ascontiguousarray(wt_e.reshape(CM, P).T),
            }
        )

    nc = _get_nc(C)
    from concourse.bass_utils import run_bass_kernel_spmd

    LAST_RESULTS = run_bass_kernel_spmd(
        nc, in_maps, core_ids=list(range(E)), trace=TRACE
    )

    y = np.zeros((N, D), dtype=np.float32)
    for e in range(E):
        o = np.asarray(LAST_RESULTS.results[e]["out"], dtype=np.float32)
        ids_e = ids_per_expert[e]
        y[ids_e] += o[: len(ids_e)]
    return y.reshape(B, T, D)
